# revision 4
# baseline (speedup 1.0000x reference)
"""Trainium2 Bass kernel v2 for nn_EnvEncoder (7-branch MLP + 2x LayerNorm).

Contract: kernel(**inputs) takes FULL unsharded inputs (x: [524288, 94] f32 +
small weights), returns FULL output [524288, 128] f32. Data-parallel over
8 cores (65536 rows each).

Math restructure vs v1:
  - LN2 is scale-invariant per row, so LN1's 1/std scaling cancels: with
    t = relu(hr - mu1) (hr = relu(mm1)), the LN2 input is
    z = t @ Wc + std1 * bc  (Wc = row-centered w_fuse, bc = centered b_fuse),
    because u = t/std1 and the un-scaled bias folds in as an extra matmul
    column carrying std1 = sqrt(var1 + eps). LN2's eps is absorbed with
    ~1e-4 relative error (<< 2e-2 tolerance).
  - mean(z) == 0 exactly (row-centered Wc/bc), so LN2 needs only E[z^2].
  - bn_stats is used in grouped form ([P, T, F] -> [P, T, 6]); the even/odd
    split stats are combined with a few batched [P, SG] ops per supergroup
    (no per-tile bn_aggr).
Engine budget per 128-row tile: PE mm1 + 2 transposes + 2 mm2 matmuls;
DVE grouped bn_stats x2 + 4x-mode affine + batched stat math; ACT relu +
transpose copy-back; finals spread over DVE/ACT/GPSIMD.
"""

import os
import numpy as np
import ml_dtypes

import concourse.bass as bass
import concourse.bacc as bacc
import concourse.tile as tile
from concourse import mybir
from concourse.bass_utils import run_bass_kernel_spmd

B_TOTAL = 524288
N_CORES = 8
B_CORE = B_TOTAL // N_CORES  # 65536
P = 128                       # samples per tile (partition dim)
K1 = 95                       # 94 features + ones row
F1 = 160                      # hidden features
F2 = 128                      # output features
T2_LO = 33                    # second transpose reads u[:, 33:161]
SG = int(os.environ.get("ENVENC_SG", "12"))  # tiles per supergroup
G1 = 3                        # mm1 outputs per PSUM bank
Q2 = 4                        # transposes/mm2 outputs per PSUM bank
LN2_BATCH = int(os.environ.get("ENVENC_LN2B", "12"))
EPS = 1e-5

_BRANCHES = [
    ("month", 0, 12, 0, 32),
    ("area", 12, 18, 32, 48),
    ("icls", 18, 24, 48, 64),
    ("scalar", 24, 26, 64, 80),
    ("long", 26, 62, 80, 112),
    ("lat", 62, 74, 112, 128),
    ("hist", 74, 94, 128, 160),
]

DT_NAME = os.environ.get("ENVENC_DT", "bfloat16")
TRACE = False

# Engine assignment knobs: "v"=DVE, "a"=ACT(scalar), "g"=gpsimd
FINAL_ENG = os.environ.get("ENVENC_FINAL", "g")
COPYBACK_ENG = os.environ.get("ENVENC_COPYBACK", "av")
AFFINE_ENG = os.environ.get("ENVENC_AFFINE", "g")
SBUFS = int(os.environ.get("ENVENC_SBUFS", "2"))      # sbuf pool depth
P1BUFS = int(os.environ.get("ENVENC_P1BUFS", "3"))
PTBUFS = int(os.environ.get("ENVENC_PTBUFS", "2"))
ZBUFS = int(os.environ.get("ENVENC_ZBUFS", "3"))
RELU_ENG = os.environ.get("ENVENC_RELU", "a")
SMALLS_ENG = os.environ.get("ENVENC_SMALLS", "g")
OUT_DMA_ENG = os.environ.get("ENVENC_ODMA", "sync")
ZCOPY_ENG = os.environ.get("ENVENC_ZCOPY", "a")

_PROGRAM_CACHE = {}
LAST_RESULTS = None


def _np_dt(dt_name):
    return np.float32 if dt_name == "float32" else ml_dtypes.bfloat16


def _my_dt(dt_name):
    return mybir.dt.float32 if dt_name == "float32" else mybir.dt.bfloat16


def _iter_chunks(n, size):
    out = []
    i = 0
    while i < n:
        out.append((i, min(size, n - i)))
        i += size
    return out


def build_program(n_tiles, dt_name, general_ln1=False, general_ln2=False):
    dt = _my_dt(dt_name)
    f32 = mybir.dt.float32
    FRelu = mybir.ActivationFunctionType.Relu
    FSqrt = mybir.ActivationFunctionType.Sqrt
    FCopy = mybir.ActivationFunctionType.Copy
    mult = mybir.AluOpType.mult
    add = mybir.AluOpType.add
    sub = mybir.AluOpType.subtract
    amax = mybir.AluOpType.max

    n_rows = n_tiles * P

    nc = bacc.Bacc("TRN2", target_bir_lowering=False, debug=False,
                   num_devices=N_CORES)

    NB = 64  # mm2b contraction size (u cols 97:161; rows 0:31 zero-padded)

    xT = nc.dram_tensor("xT", [K1, n_rows], dt, kind="ExternalInput").ap()
    w1 = nc.dram_tensor("w1", [K1, F1], dt, kind="ExternalInput").ap()
    w2a = nc.dram_tensor("w2a", [F2, F2], dt, kind="ExternalInput").ap()
    w2b = nc.dram_tensor("w2b", [NB, F2], dt, kind="ExternalInput").ap()
    ident = nc.dram_tensor("ident", [P, P], dt, kind="ExternalInput").ap()
    if general_ln1:
        g1t = nc.dram_tensor("g1t", [P, F1], dt, kind="ExternalInput").ap()
        b1t = nc.dram_tensor("b1t", [P, F1], dt, kind="ExternalInput").ap()
    if general_ln2:
        g2t = nc.dram_tensor("g2t", [P, F2], f32, kind="ExternalInput").ap()
        b2t = nc.dram_tensor("b2t", [P, F2], f32, kind="ExternalInput").ap()
    out = nc.dram_tensor("out", [n_rows, F2], dt, kind="ExternalOutput").ap()
    out_r = out.rearrange("(t p) f -> p t f", p=P)

    with tile.TileContext(nc) as tc:
        with (
            tc.tile_pool(name="consts", bufs=1) as cpool,
            tc.tile_pool(name="xc", bufs=SBUFS) as xpool,
            tc.tile_pool(name="psum1", bufs=P1BUFS, space="PSUM") as p1pool,
            tc.tile_pool(name="hr", bufs=SBUFS) as hrpool,
            tc.tile_pool(name="stats", bufs=SBUFS) as stpool,
            tc.tile_pool(name="u", bufs=SBUFS) as upool,
            tc.tile_pool(name="psumT", bufs=PTBUFS, space="PSUM") as pTpool,
            tc.tile_pool(name="uT", bufs=SBUFS + 2) as uTpool,
            tc.tile_pool(name="psum2", bufs=ZBUFS, space="PSUM") as p2pool,
            tc.tile_pool(name="st2", bufs=SBUFS + 2) as st2pool,
            tc.tile_pool(name="outb", bufs=SBUFS) as opool,
        ):
            # --- persistent constants ---
            w1_t = cpool.tile([K1, F1], dt, tag="w1")
            nc.sync.dma_start(w1_t[:], w1)
            w2a_t = cpool.tile([F2, F2], dt, tag="w2a")
            nc.sync.dma_start(w2a_t[:], w2a)
            # w2b lives at partitions 64:128 to match mm2b's lhsT base
            w2b_t = cpool.tile([P, F2], dt, tag="w2b")
            nc.sync.dma_start(w2b_t[P - NB:P, :], w2b)
            id_t = cpool.tile([P, P], dt, tag="ident")
            nc.sync.dma_start(id_t[:], ident)
            if general_ln1:
                g1_t = cpool.tile([P, F1], dt, tag="g1t")
                nc.sync.dma_start(g1_t[:], g1t)
                b1_t = cpool.tile([P, F1], dt, tag="b1t")
                nc.sync.dma_start(b1_t[:], b1t)
            if general_ln2:
                g2_t = cpool.tile([P, F2], f32, tag="g2t")
                nc.sync.dma_start(g2_t[:], g2t)
                b2_t = cpool.tile([P, F2], f32, tag="b2t")
                nc.sync.dma_start(b2_t[:], b2t)

            for sg0, sg_n in _iter_chunks(n_tiles, SG):
                xc = xpool.tile([K1, SG * P], dt, tag="xc")
                nc.sync.dma_start(xc[:, 0:sg_n * P],
                                  xT[:, sg0 * P:(sg0 + sg_n) * P])

                # --- mm1 + relu + grouped LN1 stats ---
                # hr stride-padded to F1P so grouped APs stay 3D (the interp
                # and HW honor group structure only for non-mergeable APs)
                F1P = F1 + 8
                hr = hrpool.tile([P, SG, F1P], dt, tag="hr")
                mv1 = stpool.tile([P, SG, 8], f32, tag="mv1")
                for g0, g_n in _iter_chunks(sg_n, G1):
                    p1 = p1pool.tile([P, G1 * F1], f32, tag="p1")
                    for i in range(g_n):
                        nc.tensor.matmul(
                            p1[:, i * F1:(i + 1) * F1],
                            lhsT=xc[:, (g0 + i) * P:(g0 + i + 1) * P],
                            rhs=w1_t[:],
                            start=True, stop=True,
                        )
                    p1v = p1[:, 0:g_n * F1].rearrange("p (t f) -> p t f",
                                                      f=F1)
                    if RELU_ENG == "a":
                        nc.scalar.activation(hr[:, g0:g0 + g_n, 0:F1], p1v,
                                             FRelu)
                    elif RELU_ENG == "v":
                        nc.vector.tensor_scalar(hr[:, g0:g0 + g_n, 0:F1],
                                                p1v, 0.0, None, amax)
                    else:
                        nc.gpsimd.tensor_scalar(hr[:, g0:g0 + g_n, 0:F1],
                                                p1v, 0.0, None, amax)
                    for i in range(g_n):
                        nc.vector.bn_stats(mv1[:, g0 + i, 0:6],
                                           hr[:, g0 + i, 0:F1])

                # --- batched LN1 stat combine (even/odd halves, equal n) ---
                # exact: var = (cv_e+cv_o)/F1 + (m_e-m_o)^2/4. The cross
                # term is ~var/160 and std1 only feeds the bias column, so
                # it is dropped (<0.5% std error, ~1e-4 output rel error).
                sm = nc.gpsimd if SMALLS_ENG == "g" else nc.vector
                m_e = mv1[:, 0:sg_n, 1]
                m_o = mv1[:, 0:sg_n, 4]
                cv_e = mv1[:, 0:sg_n, 2]
                cv_o = mv1[:, 0:sg_n, 5]
                musum = stpool.tile([P, SG], f32, tag="musum")
                sm.tensor_tensor(musum[:, 0:sg_n], m_e, m_o, add)
                mu1 = stpool.tile([P, SG], f32, tag="mu1")
                sm.tensor_scalar(mu1[:, 0:sg_n], musum[:, 0:sg_n],
                                 0.5, None, mult)
                vsum = stpool.tile([P, SG], f32, tag="vsum")
                sm.tensor_tensor(vsum[:, 0:sg_n], cv_e, cv_o, add)
                veps = stpool.tile([P, SG], f32, tag="veps")
                sm.tensor_scalar(veps[:, 0:sg_n], vsum[:, 0:sg_n],
                                 1.0 / F1, EPS, mult, op1=add)
                std1 = stpool.tile([P, SG], f32, tag="std1")
                nc.scalar.activation(std1[:, 0:sg_n], veps[:, 0:sg_n], FSqrt)

                # --- affine: u = [relu(hr - mu1) | std1] per tile ---
                u = upool.tile([P, SG, F1 + 1], dt, tag="u")
                sm.tensor_copy(u[:, 0:sg_n, F1:F1 + 1],
                               std1[:, 0:sg_n].rearrange(
                                   "p (t o) -> p t o", o=1))
                for i in range(sg_n):
                    if general_ln1:
                        a1 = stpool.tile([P, F1], f32, tag="a1")
                        nc.vector.tensor_scalar(
                            a1[:], hr[:, i, 0:F1],
                            mu1[:, i:i + 1], None, sub)
                        a2 = stpool.tile([P, F1], f32, tag="a2")
                        nc.vector.tensor_tensor(a2[:], a1[:], g1_t[:], mult)
                        a3 = stpool.tile([P, F1], f32, tag="a3")
                        nc.vector.scalar_tensor_tensor(
                            a3[:], b1_t[:], std1[:, i:i + 1], a2[:],
                            mult, add)
                        nc.vector.tensor_scalar(
                            u[:, i, 0:F1], a3[:], 0.0, None, amax)
                    else:
                        aeng = AFFINE_ENG[i % len(AFFINE_ENG)]
                        if aeng == "v":
                            nc.vector.tensor_scalar(
                                u[:, i, 0:F1], hr[:, i, 0:F1],
                                mu1[:, i:i + 1], 0.0, sub, op1=amax)
                        else:
                            nc.gpsimd.tensor_scalar(
                                u[:, i, 0:F1], hr[:, i, 0:F1],
                                mu1[:, i:i + 1], 0.0, sub, op1=amax)

                # --- transpose + mm2 + grouped LN2 stats + finals ---
                outb = opool.tile([P, SG, F2], dt, tag="outb")
                mv2 = st2pool.tile([P, SG, 8], f32, tag="mv2")
                zs = {}          # q0 -> psum tile
                ln2_pend = []    # q0 values awaiting rstd2
                done_upto = 0

                def flush_ln2(pend, outb=outb, mv2=mv2, zs=zs):
                    """Compute rstd2 for pending q-groups, emit finals."""
                    if not pend:
                        return
                    lo = pend[0]
                    hi = min(pend[-1] + Q2, sg_n)
                    n = hi - lo
                    sm = nc.gpsimd if SMALLS_ENG == "g" else nc.vector
                    s_e = mv2[:, lo:hi, 1]
                    s_o = mv2[:, lo:hi, 4]
                    c_e = mv2[:, lo:hi, 2]
                    c_o = mv2[:, lo:hi, 5]
                    # mean(z) == 0 exactly (centered W2), so s_e == -s_o and
                    # E[z^2] = (c_e + c_o)/F2 + s_e^2
                    t1 = st2pool.tile([P, LN2_BATCH], f32, tag="t1")
                    sm.tensor_tensor(t1[:, 0:n], c_e, c_o, add)
                    t2 = st2pool.tile([P, LN2_BATCH], f32, tag="t2")
                    sm.tensor_scalar(t2[:, 0:n], t1[:, 0:n],
                                     1.0 / F2, EPS, mult, op1=add)
                    t3 = st2pool.tile([P, LN2_BATCH], f32, tag="t3")
                    sm.tensor_tensor(t3[:, 0:n], s_e, s_o, mult)
                    v2 = st2pool.tile([P, LN2_BATCH], f32, tag="v2")
                    # E[z^2] = t2 - s_e*s_o  (s_e*s_o = -s_e^2)
                    sm.tensor_tensor(v2[:, 0:n], t2[:, 0:n], t3[:, 0:n], sub)
                    rec = st2pool.tile([P, LN2_BATCH], f32, tag="rec")
                    nc.vector.reciprocal(rec[:, 0:n], v2[:, 0:n])
                    rstd2 = st2pool.tile([P, LN2_BATCH], f32, tag="rstd2")
                    nc.scalar.activation(rstd2[:, 0:n], rec[:, 0:n], FSqrt)
                    # finals
                    k = 0
                    for q0 in pend:
                        zsb = zs.pop(q0)
                        qn = min(Q2, sg_n - q0)
                        for i in range(qn):
                            zsl = zsb[:, i, :]
                            r = rstd2[:, k:k + 1]
                            k += 1
                            ti = q0 + i
                            if general_ln2:
                                c1 = st2pool.tile([P, F2], f32, tag="c1")
                                nc.vector.tensor_scalar(c1[:], zsl, r, None,
                                                        mult)
                                c2 = st2pool.tile([P, F2], f32, tag="c2")
                                nc.vector.tensor_tensor(c2[:], c1[:], g2_t[:],
                                                        mult)
                                c3 = st2pool.tile([P, F2], f32, tag="c3")
                                nc.vector.tensor_tensor(c3[:], c2[:], b2_t[:],
                                                        add)
                                nc.vector.tensor_scalar(outb[:, ti, :], c3[:],
                                                        0.0, None, amax)
                            else:
                                eng = FINAL_ENG[ti % len(FINAL_ENG)]
                                if eng == "v":
                                    nc.vector.tensor_scalar(
                                        outb[:, ti, :], zsl, r, 0.0, mult,
                                        op1=amax)
                                elif eng == "a":
                                    nc.scalar.activation(
                                        outb[:, ti, :], zsl, FRelu, scale=r)
                                else:
                                    nc.gpsimd.tensor_scalar(
                                        outb[:, ti, :], zsl, r, 0.0, mult,
                                        op1=amax)

                for q0, q_n in _iter_chunks(sg_n, Q2):
                    pT = pTpool.tile([P, Q2, 2 * P], dt, tag="pT")
                    for i in range(q_n):
                        nc.tensor.transpose(pT[:, i, 0:P],
                                            u[:, q0 + i, 0:P], id_t[:])
                        nc.tensor.transpose(pT[:, i, P:2 * P],
                                            u[:, q0 + i, T2_LO:F1 + 1],
                                            id_t[:])
                    uT = uTpool.tile([P, Q2, 2 * P], dt, tag="uT")
                    cbe = COPYBACK_ENG[(q0 // Q2) % len(COPYBACK_ENG)]
                    if cbe == "a":
                        nc.scalar.activation(uT[:, 0:q_n, :], pT[:, 0:q_n, :],
                                             FCopy)
                    elif cbe == "v":
                        nc.vector.tensor_copy(uT[:, 0:q_n, :], pT[:, 0:q_n, :])
                    else:
                        nc.gpsimd.tensor_copy(uT[:, 0:q_n, :], pT[:, 0:q_n, :])
                    zq = p2pool.tile([P, Q2, F2], f32, tag="z")
                    for i in range(q_n):
                        nc.tensor.matmul(zq[:, i, :],
                                         lhsT=uT[:, i, 0:P],
                                         rhs=w2a_t[:],
                                         start=True, stop=False)
                        nc.tensor.matmul(zq[:, i, :],
                                         lhsT=uT[P - NB:P, i, P:2 * P],
                                         rhs=w2b_t[P - NB:P, :],
                                         start=False, stop=True)
                    zsb = uTpool.tile([P, Q2, F2], dt, tag="zsb")
                    if ZCOPY_ENG == "a":
                        nc.scalar.activation(zsb[:, 0:q_n, :], zq[:, 0:q_n, :],
                                             FCopy)
                    else:
                        nc.vector.tensor_copy(zsb[:, 0:q_n, :],
                                              zq[:, 0:q_n, :])
                    for i in range(q_n):
                        nc.vector.bn_stats(mv2[:, q0 + i, 0:6],
                                           zsb[:, i, :])
                    zs[q0] = zsb
                    ln2_pend.append(q0)
                    if (q0 + q_n) - ln2_pend[0] >= LN2_BATCH:
                        flush_ln2(ln2_pend)
                        ln2_pend = []
                flush_ln2(ln2_pend)

                odma = getattr(nc, OUT_DMA_ENG)
                odma.dma_start(out_r[:, sg0:sg0 + sg_n, :],
                               outb[:, 0:sg_n, :])

    nc.compile()
    return nc


def _prep_host(inputs, dt_name):
    ndt = _np_dt(dt_name)
    x = np.asarray(inputs["x"], np.float32)
    assert x.shape == (B_TOTAL, 94), x.shape

    w1 = np.zeros((K1, F1), np.float32)
    for name, il, ih, ol, oh in _BRANCHES:
        w1[il:ih, ol:oh] = np.asarray(inputs[f"w_{name}"], np.float32)
        w1[94, ol:oh] = np.asarray(inputs[f"b_{name}"], np.float32)

    ln1_g = np.asarray(inputs["ln1_g"], np.float32)
    ln1_b = np.asarray(inputs["ln1_b"], np.float32)
    ln2_g = np.asarray(inputs["ln2_g"], np.float32)
    ln2_b = np.asarray(inputs["ln2_b"], np.float32)
    general_ln1 = not (np.allclose(ln1_g, 1.0) and np.allclose(ln1_b, 0.0))
    general_ln2 = not (np.allclose(ln2_g, 1.0) and np.allclose(ln2_b, 0.0))

    wf = np.asarray(inputs["w_fuse"], np.float32)
    bf = np.asarray(inputs["b_fuse"], np.float32)
    wc = wf - wf.mean(axis=1, keepdims=True)
    bc = bf - bf.mean()
    w2 = np.concatenate([wc, bc[None, :]], axis=0)  # [161, 128]
    # mm2b lhsT rows map to u cols 97:161; features 97:128 are covered by
    # mm2a, so those rows are zero.
    w2b = np.zeros((64, F2), np.float32)
    w2b[31:64] = w2[F2:F1 + 1]

    xT = np.empty((K1, B_TOTAL), np.float32)
    xT[0:94] = x.T
    xT[94] = 1.0

    ident = np.eye(P, dtype=np.float32)

    core_maps = []
    for c in range(N_CORES):
        m = {
            "xT": np.ascontiguousarray(
                xT[:, c * B_CORE:(c + 1) * B_CORE]).astype(ndt),
            "w1": w1.astype(ndt),
            "w2a": np.ascontiguousarray(w2[0:F2]).astype(ndt),
            "w2b": w2b.astype(ndt),
            "ident": ident.astype(ndt),
        }
        if general_ln1:
            m["g1t"] = np.tile(ln1_g[None, :], (P, 1)).astype(ndt)
            m["b1t"] = np.tile(ln1_b[None, :], (P, 1)).astype(ndt)
        if general_ln2:
            m["g2t"] = np.tile(ln2_g[None, :], (P, 1)).astype(np.float32)
            m["b2t"] = np.tile(ln2_b[None, :], (P, 1)).astype(np.float32)
        core_maps.append(m)
    return core_maps, general_ln1, general_ln2


def kernel(**inputs):
    global LAST_RESULTS
    core_maps, gl1, gl2 = _prep_host(inputs, DT_NAME)
    key = (DT_NAME, B_CORE // P, gl1, gl2)
    if key not in _PROGRAM_CACHE:
        _PROGRAM_CACHE[key] = build_program(B_CORE // P, DT_NAME, gl1, gl2)
    nc = _PROGRAM_CACHE[key]

    res = run_bass_kernel_spmd(nc, core_maps, list(range(N_CORES)),
                               trace=TRACE)
    LAST_RESULTS = res
    out = np.empty((B_TOTAL, F2), np.float32)
    for c in range(N_CORES):
        out[c * B_CORE:(c + 1) * B_CORE] = np.asarray(
            res.results[c]["out"], dtype=np.float32)
    return out


# revision 8
# speedup vs baseline: 4.5194x; 4.5194x over previous
"""Trainium2 Bass kernel for nn_EnvEncoder (7-branch MLP + 2x LayerNorm).

Contract: kernel(**inputs) takes FULL unsharded inputs (x: [524288, 94] f32 +
small weights), returns FULL output [524288, 128] f32. Data-parallel over
8 cores (65536 rows each).

Math:
  - W1 [95,160] = block-diagonal branch weights + bias row; x is transposed
    and augmented with a ones row on the host, so mm1 = xT.T @ W1 fuses all
    seven branch Linears (+bias); relu on the way out of PSUM.
  - LN2 is scale-invariant per row, so LN1's 1/std scaling cancels: with
    t = relu(hr - mu1), the LN2 input is z = t @ Wc + std1 * bc (Wc =
    row-centered w_fuse, bc = centered b_fuse). The un-scaled bias folds in
    as an extra matmul column carrying std1 = sqrt(var1 + eps).
  - mean(z) == 0 exactly (row-centered Wc/bc), so LN2 needs only E[z^2].
  - bn_stats even/odd split stats are combined with batched [P, SG] ops
    (no bn_aggr); LN1 var drops the (m_e-m_o)^2/4 cross term (~var/160,
    std1 only feeds the bias column).
  - t is written as two tiles: ua = cols 0:128, ub = [cols 128:160 | std1 |
    zero pad]. Both are transposed 12 tiles at a time by the xbar DMA
    transpose (SBUF->SBUF, no PE, no PSUM copy-back), feeding mm2's lhsT.
  - final = relu(z * rstd2) straight into bf16 outb; host upcasts to f32.
"""

import os
import numpy as np
import ml_dtypes

import concourse.bass as bass
import concourse.bacc as bacc
import concourse.tile as tile
from concourse import mybir
from concourse.bass_utils import run_bass_kernel_spmd

B_TOTAL = 524288
N_CORES = 8
B_CORE = B_TOTAL // N_CORES  # 65536
P = 128                       # samples per tile (partition dim)
K1 = 95                       # 94 features + ones row
F1 = 160                      # hidden features
F2 = 128                      # output features
T2_LO = 33                    # PE-mode: second transpose reads u[:, 33:161]
SG = int(os.environ.get("ENVENC_SG", "24"))  # tiles per supergroup
G1 = 3                        # mm1 outputs per PSUM bank
Q2 = 4                        # mm2 outputs per PSUM bank
LN2_BATCH = int(os.environ.get("ENVENC_LN2B", "12"))
EPS = 1e-5

_BRANCHES = [
    ("month", 0, 12, 0, 32),
    ("area", 12, 18, 32, 48),
    ("icls", 18, 24, 48, 64),
    ("scalar", 24, 26, 64, 80),
    ("long", 26, 62, 80, 112),
    ("lat", 62, 74, 112, 128),
    ("hist", 74, 94, 128, 160),
]

DT_NAME = os.environ.get("ENVENC_DT", "bfloat16")
TRACE = False

# Engine assignment knobs: "v"=DVE, "a"=ACT(scalar)
FINAL_ENG = os.environ.get("ENVENC_FINAL", "av")
RELU_ENG = os.environ.get("ENVENC_RELU", "a")
ZCOPY_ENG = os.environ.get("ENVENC_ZCOPY", "a")
UBAFF_ENG = os.environ.get("ENVENC_UBAFF", "v")
TMODE = os.environ.get("ENVENC_TMODE", "dma")   # "dma" xbar / "pe" matmul
COPYBACK_ENG = os.environ.get("ENVENC_COPYBACK", "av")  # pe-mode only
SBUFS = int(os.environ.get("ENVENC_SBUFS", "2"))
P1BUFS = int(os.environ.get("ENVENC_P1BUFS", "3"))
ZBUFS = int(os.environ.get("ENVENC_ZBUFS", "4"))
TDMA_ENG = os.environ.get("ENVENC_TDMA", "sync")
OUT_DMA_ENG = os.environ.get("ENVENC_ODMA", "sync")

_PROGRAM_CACHE = {}
LAST_RESULTS = None


def _np_dt(dt_name):
    return np.float32 if dt_name == "float32" else ml_dtypes.bfloat16


def _my_dt(dt_name):
    return mybir.dt.float32 if dt_name == "float32" else mybir.dt.bfloat16


def _iter_chunks(n, size):
    out = []
    i = 0
    while i < n:
        out.append((i, min(size, n - i)))
        i += size
    return out


def build_program(n_tiles, dt_name, general_ln1=False, general_ln2=False):
    dt = _my_dt(dt_name)
    f32 = mybir.dt.float32
    FRelu = mybir.ActivationFunctionType.Relu
    FSqrt = mybir.ActivationFunctionType.Sqrt
    FCopy = mybir.ActivationFunctionType.Copy
    mult = mybir.AluOpType.mult
    add = mybir.AluOpType.add
    sub = mybir.AluOpType.subtract
    amax = mybir.AluOpType.max

    n_rows = n_tiles * P
    dma_t = TMODE == "dma"

    nc = bacc.Bacc("TRN2", target_bir_lowering=False, debug=False,
                   num_devices=N_CORES)

    NB = 64   # mm2b contraction size
    F1P = F1 + 8   # hr row stride (pads per-tile slices to 4B alignment)
    F1U = F1 + 2   # pe-mode u row stride

    xT = nc.dram_tensor("xT", [K1, n_rows], dt, kind="ExternalInput").ap()
    w1 = nc.dram_tensor("w1", [K1, F1], dt, kind="ExternalInput").ap()
    w2a = nc.dram_tensor("w2a", [F2, F2], dt, kind="ExternalInput").ap()
    w2b = nc.dram_tensor("w2b", [NB, F2], dt, kind="ExternalInput").ap()
    ident = nc.dram_tensor("ident", [P, P], dt, kind="ExternalInput").ap()
    if general_ln1:
        g1t = nc.dram_tensor("g1t", [P, F1], dt, kind="ExternalInput").ap()
        b1t = nc.dram_tensor("b1t", [P, F1], dt, kind="ExternalInput").ap()
    if general_ln2:
        g2t = nc.dram_tensor("g2t", [P, F2], f32, kind="ExternalInput").ap()
        b2t = nc.dram_tensor("b2t", [P, F2], f32, kind="ExternalInput").ap()
    out = nc.dram_tensor("out", [n_rows, F2], dt, kind="ExternalOutput").ap()
    out_r = out.rearrange("(t p) f -> p t f", p=P)

    with tile.TileContext(nc) as tc:
        with (
            tc.tile_pool(name="consts", bufs=1) as cpool,
            tc.tile_pool(name="xc", bufs=SBUFS) as xpool,
            tc.tile_pool(name="psum1", bufs=P1BUFS, space="PSUM") as p1pool,
            tc.tile_pool(name="hr", bufs=SBUFS) as hrpool,
            tc.tile_pool(name="stats", bufs=SBUFS) as stpool,
            tc.tile_pool(name="u", bufs=SBUFS) as upool,
            tc.tile_pool(name="psumT", bufs=(1 if dma_t else 2),
                         space="PSUM") as pTpool,
            tc.tile_pool(name="uT", bufs=SBUFS + 2) as uTpool,
            tc.tile_pool(name="psum2", bufs=ZBUFS, space="PSUM") as p2pool,
            tc.tile_pool(name="st2", bufs=SBUFS + 2) as st2pool,
            tc.tile_pool(name="outb", bufs=SBUFS) as opool,
        ):
            # --- persistent constants ---
            w1_t = cpool.tile([K1, F1], dt, tag="w1")
            nc.sync.dma_start(w1_t[:], w1)
            w2a_t = cpool.tile([F2, F2], dt, tag="w2a")
            nc.sync.dma_start(w2a_t[:], w2a)
            if dma_t:
                # dma mode: w2b rows = [feats 128:160 | bias | zeros], base 0
                w2b_t = cpool.tile([NB, F2], dt, tag="w2b")
                nc.sync.dma_start(w2b_t[:], w2b)
            else:
                # pe mode: w2b lives at partitions 64:128 (lhsT base 64)
                w2b_t = cpool.tile([P, F2], dt, tag="w2b")
                nc.sync.dma_start(w2b_t[P - NB:P, :], w2b)
            id_t = cpool.tile([P, P], dt, tag="ident")
            nc.sync.dma_start(id_t[:], ident)
            if general_ln1:
                g1_t = cpool.tile([P, F1], dt, tag="g1t")
                nc.sync.dma_start(g1_t[:], g1t)
                b1_t = cpool.tile([P, F1], dt, tag="b1t")
                nc.sync.dma_start(b1_t[:], b1t)
            if general_ln2:
                g2_t = cpool.tile([P, F2], f32, tag="g2t")
                nc.sync.dma_start(g2_t[:], g2t)
                b2_t = cpool.tile([P, F2], f32, tag="b2t")
                nc.sync.dma_start(b2_t[:], b2t)

            sm = nc.vector
            tdma = getattr(nc, TDMA_ENG)

            if dma_t:
                # manual double-buffered ub tiles; zero pad columns once
                # (uTb rows 33:128 multiply w2b's zero rows in mm2b)
                ub_tiles = []
                for r in range(2):
                    ubt = cpool.tile([P, SG, P], dt, tag=f"ub{r}")
                    nc.vector.memset(ubt[:], 0.0)
                    ub_tiles.append(ubt)

            def emit_front(sg0, sg_n, sg_i):
                """DMA in, mm1, relu, LN1 stats + combine for one SG."""
                xc = xpool.tile([K1, SG * P], dt, tag="xc")
                nc.sync.dma_start(xc[:, 0:sg_n * P],
                                  xT[:, sg0 * P:(sg0 + sg_n) * P])
                hr = hrpool.tile([P, SG, F1P], dt, tag="hr")
                mv1 = stpool.tile([P, SG, 8], f32, tag="mv1")
                for g0, g_n in _iter_chunks(sg_n, G1):
                    p1 = p1pool.tile([P, G1 * F1], f32, tag="p1")
                    for i in range(g_n):
                        nc.tensor.matmul(
                            p1[:, i * F1:(i + 1) * F1],
                            lhsT=xc[:, (g0 + i) * P:(g0 + i + 1) * P],
                            rhs=w1_t[:],
                            start=True, stop=True,
                        )
                    p1v = p1[:, 0:g_n * F1].rearrange("p (t f) -> p t f",
                                                      f=F1)
                    if RELU_ENG == "a":
                        nc.scalar.activation(hr[:, g0:g0 + g_n, 0:F1], p1v,
                                             FRelu)
                    else:
                        nc.vector.tensor_scalar(hr[:, g0:g0 + g_n, 0:F1],
                                                p1v, 0.0, None, amax)
                    for i in range(g_n):
                        nc.vector.bn_stats(mv1[:, g0 + i, 0:6],
                                           hr[:, g0 + i, 0:F1])

                # batched LN1 stat combine. Exact var has a (m_e-m_o)^2/4
                # cross term (~var/160); std1 only feeds the bias column,
                # so it is dropped (~1e-4 output rel error).
                m_e = mv1[:, 0:sg_n, 1]
                m_o = mv1[:, 0:sg_n, 4]
                cv_e = mv1[:, 0:sg_n, 2]
                cv_o = mv1[:, 0:sg_n, 5]
                musum = stpool.tile([P, SG], f32, tag="musum")
                sm.tensor_tensor(musum[:, 0:sg_n], m_e, m_o, add)
                mu1 = stpool.tile([P, SG], f32, tag="mu1")
                sm.tensor_scalar(mu1[:, 0:sg_n], musum[:, 0:sg_n],
                                 0.5, None, mult)
                vsum = stpool.tile([P, SG], f32, tag="vsum")
                sm.tensor_tensor(vsum[:, 0:sg_n], cv_e, cv_o, add)
                veps = stpool.tile([P, SG], f32, tag="veps")
                sm.tensor_scalar(veps[:, 0:sg_n], vsum[:, 0:sg_n],
                                 1.0 / F1, EPS, mult, op1=add)
                std1 = stpool.tile([P, SG], f32, tag="std1")
                nc.scalar.activation(std1[:, 0:sg_n], veps[:, 0:sg_n], FSqrt)
                return (sg0, sg_n, sg_i, hr, mu1, std1, veps)

            def affine_general(u_sl, hr_sl, mu_sc, std_sc):
                a1 = stpool.tile([P, F1], f32, tag="a1")
                nc.vector.tensor_scalar(a1[:], hr_sl, mu_sc, None, sub)
                a2 = stpool.tile([P, F1], f32, tag="a2")
                nc.vector.tensor_tensor(a2[:], a1[:], g1_t[:], mult)
                a3 = stpool.tile([P, F1], f32, tag="a3")
                nc.vector.scalar_tensor_tensor(a3[:], b1_t[:], std_sc,
                                               a2[:], mult, add)
                nc.vector.tensor_scalar(u_sl, a3[:, 0:u_sl.shape[-1]],
                                        0.0, None, amax)
                return a3

            def finals(pend_z, rstd2, outb, k0=0):
                k = k0
                for (ti, zsl) in pend_z:
                    r = rstd2[:, k:k + 1]
                    k += 1
                    if general_ln2:
                        c1 = st2pool.tile([P, F2], f32, tag="c1")
                        nc.vector.tensor_scalar(c1[:], zsl, r, None, mult)
                        c2 = st2pool.tile([P, F2], f32, tag="c2")
                        nc.vector.tensor_tensor(c2[:], c1[:], g2_t[:], mult)
                        c3 = st2pool.tile([P, F2], f32, tag="c3")
                        nc.vector.tensor_tensor(c3[:], c2[:], b2_t[:], add)
                        nc.vector.tensor_scalar(outb[:, ti, :], c3[:],
                                                0.0, None, amax)
                    else:
                        eng = FINAL_ENG[ti % len(FINAL_ENG)]
                        if eng == "v":
                            nc.vector.tensor_scalar(outb[:, ti, :], zsl, r,
                                                    0.0, mult, op1=amax)
                        else:
                            nc.scalar.activation(outb[:, ti, :], zsl, FRelu,
                                                 scale=r)

            def ln2_math(mv2, lo, n):
                s_e = mv2[:, lo:lo + n, 1]
                s_o = mv2[:, lo:lo + n, 4]
                c_e = mv2[:, lo:lo + n, 2]
                c_o = mv2[:, lo:lo + n, 5]
                # mean(z) == 0 exactly (centered W2): s_e == -s_o and
                # E[z^2] = (c_e + c_o)/F2 + s_e^2 = t2 - s_e*s_o
                t1 = st2pool.tile([P, LN2_BATCH], f32, tag="t1")
                sm.tensor_tensor(t1[:, 0:n], c_e, c_o, add)
                t2 = st2pool.tile([P, LN2_BATCH], f32, tag="t2")
                sm.tensor_scalar(t2[:, 0:n], t1[:, 0:n],
                                 1.0 / F2, EPS, mult, op1=add)
                t3 = st2pool.tile([P, LN2_BATCH], f32, tag="t3")
                sm.tensor_tensor(t3[:, 0:n], s_e, s_o, mult)
                v2 = st2pool.tile([P, LN2_BATCH], f32, tag="v2")
                sm.tensor_tensor(v2[:, 0:n], t2[:, 0:n], t3[:, 0:n], sub)
                rec = st2pool.tile([P, LN2_BATCH], f32, tag="rec")
                nc.vector.reciprocal(rec[:, 0:n], v2[:, 0:n])
                rstd2 = st2pool.tile([P, LN2_BATCH], f32, tag="rstd2")
                nc.scalar.activation(rstd2[:, 0:n], rec[:, 0:n], FSqrt)
                return rstd2

            def emit_back_dma(state):
                sg0, sg_n, sg_i, hr, mu1, std1, veps = state
                ua = upool.tile([P, SG, P], dt, tag="ua")
                ub = ub_tiles[sg_i % 2]
                # std1 straight into ub col 32 (bias column after transpose)
                nc.scalar.activation(
                    ub[:, 0:sg_n, 32:33],
                    veps[:, 0:sg_n].rearrange("p (t o) -> p t o", o=1),
                    FSqrt)
                for i in range(sg_n):
                    if general_ln1:
                        a3 = affine_general(ua[:, i, :], hr[:, i, 0:F1],
                                            mu1[:, i:i + 1], std1[:, i:i + 1])
                        nc.vector.tensor_scalar(ub[:, i, 0:32],
                                                a3[:, P:F1], 0.0, None, amax)
                    else:
                        nc.vector.tensor_scalar(
                            ua[:, i, :], hr[:, i, 0:P],
                            mu1[:, i:i + 1], 0.0, sub, op1=amax)
                        nc.vector.tensor_scalar(
                            ub[:, i, 0:32], hr[:, i, P:F1],
                            mu1[:, i:i + 1], 0.0, sub, op1=amax)

                outb = opool.tile([P, SG, F2], dt, tag="outb")
                mv2 = st2pool.tile([P, SG, 8], f32, tag="mv2")
                for lo, n in _iter_chunks(sg_n, LN2_BATCH):
                    uTa = uTpool.tile([P, LN2_BATCH, P], dt, tag="uTa")
                    uTb = uTpool.tile([P, LN2_BATCH, P], dt, tag="uTb")
                    tdma.dma_start_transpose(uTa[:, 0:n, :],
                                             ua[:, lo:lo + n, :])
                    tdma.dma_start_transpose(uTb[:, 0:n, :],
                                             ub[:, lo:lo + n, :])
                    zsbs = []
                    for q0, q_n in _iter_chunks(n, Q2):
                        zq = p2pool.tile([P, Q2, F2], f32, tag="z")
                        for i in range(q_n):
                            nc.tensor.matmul(zq[:, i, :],
                                             lhsT=uTa[:, q0 + i, :],
                                             rhs=w2a_t[:],
                                             start=True, stop=False)
                            nc.tensor.matmul(zq[:, i, :],
                                             lhsT=uTb[0:NB, q0 + i, :],
                                             rhs=w2b_t[0:NB, :],
                                             start=False, stop=True)
                        zsb = uTpool.tile([P, Q2, F2], dt, tag="zsb")
                        if ZCOPY_ENG == "a":
                            nc.scalar.activation(zsb[:, 0:q_n, :],
                                                 zq[:, 0:q_n, :], FCopy)
                        else:
                            nc.vector.tensor_copy(zsb[:, 0:q_n, :],
                                                  zq[:, 0:q_n, :])
                        for i in range(q_n):
                            nc.vector.bn_stats(mv2[:, lo + q0 + i, 0:6],
                                               zsb[:, i, :])
                        zsbs.append((q0, q_n, zsb))
                    rstd2 = ln2_math(mv2, lo, n)
                    for q0, q_n, zsb in zsbs:
                        pend_z = [(lo + q0 + i, zsb[:, i, :])
                                  for i in range(q_n)]
                        finals(pend_z, rstd2, outb, k0=q0)
                odma = getattr(nc, OUT_DMA_ENG)
                odma.dma_start(out_r[:, sg0:sg0 + sg_n, :],
                               outb[:, 0:sg_n, :])

            def emit_back_pe(state):
                sg0, sg_n, sg_i, hr, mu1, std1, veps = state
                u = upool.tile([P, SG, F1U], dt, tag="u")
                nc.scalar.activation(
                    u[:, 0:sg_n, F1:F1 + 1],
                    veps[:, 0:sg_n].rearrange("p (t o) -> p t o", o=1),
                    FSqrt)
                for i in range(sg_n):
                    if general_ln1:
                        affine_general(u[:, i, 0:F1], hr[:, i, 0:F1],
                                       mu1[:, i:i + 1], std1[:, i:i + 1])
                    else:
                        nc.vector.tensor_scalar(
                            u[:, i, 0:F1], hr[:, i, 0:F1],
                            mu1[:, i:i + 1], 0.0, sub, op1=amax)

                outb = opool.tile([P, SG, F2], dt, tag="outb")
                mv2 = st2pool.tile([P, SG, 8], f32, tag="mv2")
                for lo, n in _iter_chunks(sg_n, LN2_BATCH):
                    zsbs = []
                    for q0, q_n in _iter_chunks(n, Q2):
                        pT = pTpool.tile([P, Q2, 2 * P], dt, tag="pT")
                        for i in range(q_n):
                            nc.tensor.transpose(pT[:, i, 0:P],
                                                u[:, lo + q0 + i, 0:P],
                                                id_t[:])
                            nc.tensor.transpose(pT[:, i, P:2 * P],
                                                u[:, lo + q0 + i,
                                                  T2_LO:F1 + 1],
                                                id_t[:])
                        uT = uTpool.tile([P, Q2, 2 * P], dt, tag="uT")
                        cbe = COPYBACK_ENG[(q0 // Q2) % len(COPYBACK_ENG)]
                        if cbe == "a":
                            nc.scalar.activation(uT[:, 0:q_n, :],
                                                 pT[:, 0:q_n, :], FCopy)
                        else:
                            nc.vector.tensor_copy(uT[:, 0:q_n, :],
                                                  pT[:, 0:q_n, :])
                        zq = p2pool.tile([P, Q2, F2], f32, tag="z")
                        for i in range(q_n):
                            nc.tensor.matmul(zq[:, i, :],
                                             lhsT=uT[:, i, 0:P],
                                             rhs=w2a_t[:],
                                             start=True, stop=False)
                            nc.tensor.matmul(zq[:, i, :],
                                             lhsT=uT[P - NB:P, i, P:2 * P],
                                             rhs=w2b_t[P - NB:P, :],
                                             start=False, stop=True)
                        zsb = uTpool.tile([P, Q2, F2], dt, tag="zsb")
                        if ZCOPY_ENG == "a":
                            nc.scalar.activation(zsb[:, 0:q_n, :],
                                                 zq[:, 0:q_n, :], FCopy)
                        else:
                            nc.vector.tensor_copy(zsb[:, 0:q_n, :],
                                                  zq[:, 0:q_n, :])
                        for i in range(q_n):
                            nc.vector.bn_stats(mv2[:, lo + q0 + i, 0:6],
                                               zsb[:, i, :])
                        zsbs.append((q0, q_n, zsb))
                    rstd2 = ln2_math(mv2, lo, n)
                    for q0, q_n, zsb in zsbs:
                        pend_z = [(lo + q0 + i, zsb[:, i, :])
                                  for i in range(q_n)]
                        finals(pend_z, rstd2, outb, k0=q0)
                odma = getattr(nc, OUT_DMA_ENG)
                odma.dma_start(out_r[:, sg0:sg0 + sg_n, :],
                               outb[:, 0:sg_n, :])

            emit_back = emit_back_dma if dma_t else emit_back_pe
            for sg_i, (sg0, sg_n) in enumerate(_iter_chunks(n_tiles, SG)):
                emit_back(emit_front(sg0, sg_n, sg_i))

    nc.compile()
    return nc


def _prep_host(inputs, dt_name):
    ndt = _np_dt(dt_name)
    x = np.asarray(inputs["x"], np.float32)
    assert x.shape == (B_TOTAL, 94), x.shape

    w1 = np.zeros((K1, F1), np.float32)
    for name, il, ih, ol, oh in _BRANCHES:
        w1[il:ih, ol:oh] = np.asarray(inputs[f"w_{name}"], np.float32)
        w1[94, ol:oh] = np.asarray(inputs[f"b_{name}"], np.float32)

    ln1_g = np.asarray(inputs["ln1_g"], np.float32)
    ln1_b = np.asarray(inputs["ln1_b"], np.float32)
    ln2_g = np.asarray(inputs["ln2_g"], np.float32)
    ln2_b = np.asarray(inputs["ln2_b"], np.float32)
    general_ln1 = not (np.allclose(ln1_g, 1.0) and np.allclose(ln1_b, 0.0))
    general_ln2 = not (np.allclose(ln2_g, 1.0) and np.allclose(ln2_b, 0.0))

    wf = np.asarray(inputs["w_fuse"], np.float32)
    bf = np.asarray(inputs["b_fuse"], np.float32)
    wc = wf - wf.mean(axis=1, keepdims=True)
    bc = bf - bf.mean()
    w2 = np.concatenate([wc, bc[None, :]], axis=0)  # [161, 128]
    w2b = np.zeros((64, F2), np.float32)
    if TMODE == "dma":
        # dma mode: uTb rows = [feats 128:160 | std | zeros]
        w2b[0:33] = w2[F2:F1 + 1]
    else:
        # pe mode: uT2 rows map to u cols 97:161; first 31 rows unused
        w2b[31:64] = w2[F2:F1 + 1]

    xT = np.empty((K1, B_TOTAL), np.float32)
    xT[0:94] = x.T
    xT[94] = 1.0

    ident = np.eye(P, dtype=np.float32)

    core_maps = []
    for c in range(N_CORES):
        m = {
            "xT": np.ascontiguousarray(
                xT[:, c * B_CORE:(c + 1) * B_CORE]).astype(ndt),
            "w1": w1.astype(ndt),
            "w2a": np.ascontiguousarray(w2[0:F2]).astype(ndt),
            "w2b": w2b.astype(ndt),
            "ident": ident.astype(ndt),
        }
        if general_ln1:
            m["g1t"] = np.tile(ln1_g[None, :], (P, 1)).astype(ndt)
            m["b1t"] = np.tile(ln1_b[None, :], (P, 1)).astype(ndt)
        if general_ln2:
            m["g2t"] = np.tile(ln2_g[None, :], (P, 1)).astype(np.float32)
            m["b2t"] = np.tile(ln2_b[None, :], (P, 1)).astype(np.float32)
        core_maps.append(m)
    return core_maps, general_ln1, general_ln2


def kernel(**inputs):
    global LAST_RESULTS
    core_maps, gl1, gl2 = _prep_host(inputs, DT_NAME)
    key = (DT_NAME, B_CORE // P, gl1, gl2, TMODE)
    if key not in _PROGRAM_CACHE:
        _PROGRAM_CACHE[key] = build_program(B_CORE // P, DT_NAME, gl1, gl2)
    nc = _PROGRAM_CACHE[key]

    res = run_bass_kernel_spmd(nc, core_maps, list(range(N_CORES)),
                               trace=TRACE)
    LAST_RESULTS = res
    out = np.empty((B_TOTAL, F2), np.float32)
    for c in range(N_CORES):
        out[c * B_CORE:(c + 1) * B_CORE] = np.asarray(
            res.results[c]["out"], dtype=np.float32)
    return out
